# revision 10
# baseline (speedup 1.0000x reference)
"""
CIN (Compressed Interaction Network) kernel for Trainium2, 8 NeuronCores.

Problem (hardcoded):
  x: [4096, 32, 64] fp32; w0: [128, 1024]; b0: [128]; w1: [128, 2048]; b1: [128]
  out: [4096, 192] = concat(relu(y0)[:, 64:], relu(y1)).sum(d)

Sharding: data parallel over batch, 512 samples/core, tokens t=(b,d), T=32768.

Key structure (per core, software-pipelined over 2048-token pairs):
  - L0 via polarization: x_h*x_f = ((x_h+x_f)^2 - x_h^2 - x_f^2)/2 ->
    496 upper-triangle sum-channels + 32 squares = 528 channels (vs 1024).
    Built on the PE as K=32 two-hot matmuls (4-way tile_position
    concurrency over the 4 x-copies); ScalarE evacuates with func=Square;
    contraction uses host-folded weights.
  - L1 z-channels are PERMUTED to (f, h) order so the broadcast operand is
    built from x (a pure input): xe_g[p, t] = x[2g + p//64, t] is
    DMA-expanded from HBM with a replicating access pattern - a pure
    prefetch with no upstream dependency (no hidden round-trip). The hidden
    side is one 2x-tiled SBUF copy per pair (hd2[p] = hidden[p % 64]).
    Contract weights are host-permuted to match.
  - z muls run in place on the expanded tiles (VectorE x14, GpSimd x2);
    contracts are group-major into 4 resident PSUM quarter tiles; y evac
    ScalarE Relu+bias; d-sums via log2 trees of strided VectorE adds.
  - Block P emits: xe prefetch for pair P, L0 of pair P, then muls +
    L1 contract of pair P-1, keeping every engine a full block ahead of
    its consumers.
"""

import sys

import numpy as np
import ml_dtypes

sys.path.insert(0, "/opt/trn_rl_repo")

B_FULL = 4096
N_CORES = 8
BS = B_FULL // N_CORES  # 512
F = 32
D = 64
T = BS * D
PAIR = 2048
O = 128
H1 = 64
G1 = 16
CH0 = 528
NG0 = 5

GPS_MULS = (14, 15)

_CACHE = {}


def _build_nc(BS=BS):
    import concourse.bass as bass
    import concourse.tile as tile
    from concourse import bacc, mybir

    T = BS * D
    NPAIR = T // PAIR
    SPP = PAIR // D

    bf16 = mybir.dt.bfloat16
    f32 = mybir.dt.float32
    Relu = mybir.ActivationFunctionType.Relu
    Square = mybir.ActivationFunctionType.Square

    nc = bacc.Bacc(None, target_bir_lowering=False)

    xt = nc.dram_tensor("xt", [128, T], bf16, kind="ExternalInput")
    a0 = nc.dram_tensor("a0", [128, NG0, 128], bf16, kind="ExternalInput")
    w0f = nc.dram_tensor("w0f", [128, NG0, O], bf16, kind="ExternalInput")
    w1g = nc.dram_tensor("w1g", [128, G1, O], bf16, kind="ExternalInput")
    b0 = nc.dram_tensor("b0", [O, 1], f32, kind="ExternalInput")
    b1 = nc.dram_tensor("b1", [O, 1], f32, kind="ExternalInput")
    out0 = nc.dram_tensor("out0", [O - H1, BS], f32, kind="ExternalOutput")
    out1 = nc.dram_tensor("out1", [O, BS], f32, kind="ExternalOutput")

    with tile.TileContext(nc) as tc:
        with (
            tc.tile_pool(name="singles", bufs=1) as singles,
            tc.tile_pool(name="xrp", bufs=2) as xrp,
            tc.tile_pool(name="s0p", bufs=6) as s0p,
            tc.tile_pool(name="xep", bufs=13) as xep,
            tc.tile_pool(name="hd2p", bufs=2) as hd2p,
            tc.tile_pool(name="y0sbp", bufs=2) as y0sbp,
            tc.tile_pool(name="y1sbp", bufs=2) as y1sbp,
            tc.tile_pool(name="bcps", bufs=2, space="PSUM") as bcps,
            tc.tile_pool(name="yqp", bufs=4, space="PSUM") as yqp,
        ):
            a0s = singles.tile([128, NG0, 128], bf16)
            w0fs = singles.tile([128, NG0, O], bf16)
            w1s = singles.tile([128, G1, O], bf16)
            b0s = singles.tile([O, 1], f32)
            b1s = singles.tile([O, 1], f32)
            oacc0 = singles.tile([O, BS], f32)
            oacc1 = singles.tile([O, BS], f32)

            nc.gpsimd.dma_start(out=a0s[:], in_=a0[:])
            nc.gpsimd.dma_start(out=w0fs[:], in_=w0f[:])
            nc.gpsimd.dma_start(out=w1s[:], in_=w1g[:])
            nc.gpsimd.dma_start(out=b0s[:], in_=b0[:])
            nc.gpsimd.dma_start(out=b1s[:], in_=b1[:])

            # x rows with a broadcast-expansion view for the xe prefetches
            xb = xt.rearrange("(a h) t -> a h t", a=1).broadcast_to((64, 128, T))

            st = {}

            def prefetch_xe(P):
                """xe_g[p, t] = x[2g + p//64, t] for all 16 groups (pure input)."""
                xes = []
                for R in range(8):
                    xe = xep.tile([128, 2, PAIR], bf16, name=f"xe{P}_{R}", tag="xe")
                    xes.append(xe)
                for g in range(G1):
                    src = xb[:, 2 * g : 2 * g + 2, P * PAIR : (P + 1) * PAIR]
                    src = src.rearrange("r h t -> h r t")
                    eng = nc.sync if g < 4 else nc.gpsimd
                    eng.dma_start(out=xes[g // 2][:, g % 2, :], in_=src)
                st[P] = {"xes": xes}

            def s1_load(P):
                xr = xrp.tile([128, PAIR], bf16, name=f"xr{P}", tag="xr")
                nc.sync.dma_start(out=xr[:], in_=xt[:, P * PAIR : (P + 1) * PAIR])
                st[P]["xr"] = xr
                st[P]["s0"] = []

            def s1_build(P, G):
                xr = st[P]["xr"]
                psA = bcps.tile([128, 2, 512], f32, name=f"psA{P}_{G}", tag="bc")
                psB = bcps.tile([128, 2, 512], f32, name=f"psB{P}_{G}", tag="bc")
                for c in range(4):
                    ps = psA if c < 2 else psB
                    nc.tensor.matmul(
                        ps[:, c % 2, :],
                        a0s[32 * c : 32 * c + 32, G, :],
                        xr[32 * c : 32 * c + 32, 512 * c : 512 * (c + 1)],
                        start=True, stop=True, tile_position=(32 * c, 0),
                    )
                sg = s0p.tile([128, PAIR], bf16, name=f"s0_{P}_{G}", tag="s0")
                for half, ps in ((0, psA), (1, psB)):
                    dst = sg[:, 1024 * half : 1024 * (half + 1)].rearrange(
                        "p (j c) -> p j c", j=2
                    )
                    nc.scalar.activation(dst, ps[:], Square)
                st[P]["s0"].append(sg)

            def s2(P):
                """L0 contract + y0 evac + out0 tree + hd2 copies."""
                s0g = st[P]["s0"]
                y0sb = y0sbp.tile([128, PAIR], bf16, name=f"y0sb{P}", tag="y0sb")
                yq = [
                    yqp.tile([128, 512], f32, name=f"y0q{P}_{q}", tag="yq")
                    for q in range(4)
                ]
                for G in range(NG0):
                    for q in range(4):
                        nc.tensor.matmul(
                            yq[q][:], w0fs[:, G, :],
                            s0g[G][:, 512 * q : 512 * (q + 1)],
                            start=(G == 0), stop=(G == NG0 - 1),
                        )
                for q in range(4):
                    nc.scalar.activation(
                        y0sb[:, 512 * q : 512 * (q + 1)], yq[q][:], Relu, bias=b0s[:]
                    )
                hd2 = hd2p.tile([128, PAIR], bf16, name=f"hd2_{P}", tag="hd2")
                nc.sync.dma_start(out=hd2[0:64, :], in_=y0sb[0:H1, :])
                nc.sync.dma_start(out=hd2[64:128, :], in_=y0sb[0:H1, :])
                st[P]["hd2"] = hd2
                w = D // 2
                while w >= 1:
                    a = y0sb[H1:O, :].rearrange("p (b d) -> p b d", d=D)
                    dst = (
                        oacc0[H1:O, P * SPP : (P + 1) * SPP]
                        if w == 1
                        else a[:, :, 0:w]
                    )
                    nc.vector.tensor_add(dst, a[:, :, 0:w], a[:, :, w : 2 * w])
                    w //= 2

            def s3_muls(P):
                xes, hd2 = st[P]["xes"], st[P]["hd2"]
                for g in range(G1):
                    xeg = xes[g // 2][:, g % 2, :]
                    eng = nc.gpsimd if g in GPS_MULS else nc.vector
                    eng.tensor_mul(xeg, xeg, hd2[:])
                y1sb = y1sbp.tile([128, PAIR], bf16, name=f"y1sb{P}", tag="y1sb")
                yq = [
                    yqp.tile([128, 512], f32, name=f"y1q{P}_{q}", tag="yq")
                    for q in range(4)
                ]
                st[P]["y1sb"] = y1sb
                st[P]["yq"] = yq

            def s3_contract(P, glo, ghi):
                xes, yq = st[P]["xes"], st[P]["yq"]
                for g in range(glo, ghi):
                    for q in range(4):
                        nc.tensor.matmul(
                            yq[q][:], w1s[:, g, :],
                            xes[g // 2][:, g % 2, 512 * q : 512 * (q + 1)],
                            start=(g == 0), stop=(g == G1 - 1),
                        )

            def s3_fini(P):
                y1sb, yq = st[P]["y1sb"], st[P]["yq"]
                for q in range(4):
                    nc.scalar.activation(
                        y1sb[:, 512 * q : 512 * (q + 1)], yq[q][:], Relu, bias=b1s[:]
                    )
                w = D // 2
                while w >= 1:
                    a = y1sb[:].rearrange("p (b d) -> p b d", d=D)
                    dst = (
                        oacc1[:, P * SPP : (P + 1) * SPP] if w == 1 else a[:, :, 0:w]
                    )
                    nc.vector.tensor_add(dst, a[:, :, 0:w], a[:, :, w : 2 * w])
                    w //= 2
                del st[P]

            for P in range(NPAIR + 1):
                p1, p3 = P, P - 1
                if p1 < NPAIR:
                    prefetch_xe(p1)
                    s1_load(p1)
                    s1_build(p1, 0)
                    s1_build(p1, 1)
                if 0 <= p3:
                    s3_muls(p3)
                    s3_contract(p3, 0, 4)
                if p1 < NPAIR:
                    s1_build(p1, 2)
                if 0 <= p3:
                    s3_contract(p3, 4, 8)
                if p1 < NPAIR:
                    s1_build(p1, 3)
                if 0 <= p3:
                    s3_contract(p3, 8, 12)
                if p1 < NPAIR:
                    s1_build(p1, 4)
                if 0 <= p3:
                    s3_contract(p3, 12, 16)
                    s3_fini(p3)
                if p1 < NPAIR:
                    s2(p1)

            nc.gpsimd.dma_start(out=out0[:], in_=oacc0[H1:O, :])
            nc.gpsimd.dma_start(out=out1[:], in_=oacc1[:])

    nc.finalize()
    return nc


def _get_nc():
    if "nc" not in _CACHE:
        _CACHE["nc"] = _build_nc()
    return _CACHE["nc"]


def _l0_pairs():
    return [(h, f) for h in range(F) for f in range(h + 1, F)]


def make_l0(w0_np):
    """A0 build matrix [32, 640] and folded weights [640, 128] (zero-padded)."""
    pairs = _l0_pairs()
    A0 = np.zeros((F, NG0 * 128), np.float32)
    w0fold = np.zeros((NG0 * 128, O), np.float32)
    for k, (h, f) in enumerate(pairs):
        A0[h, k] = 1.0
        A0[f, k] = 1.0
        w0fold[k] = (w0_np[:, h * F + f] + w0_np[:, f * F + h]) / 2
    for h in range(F):
        k = 496 + h
        A0[h, k] = 1.0
        c = w0_np[:, h * F + h].copy()
        for f in range(F):
            if f != h:
                c -= 0.5 * (w0_np[:, h * F + f] + w0_np[:, f * F + h])
        w0fold[k] = c
    return A0, w0fold


def make_w1g(w1_np):
    """Permuted L1 weights [128, 16, 128]: channel (g, p) -> h*F + f with
    f = 2g + p//64, h = p%64."""
    w1t = np.ascontiguousarray(np.asarray(w1_np, dtype=np.float32).T)  # [2048, O]
    p = np.arange(128)
    out = np.empty((128, G1, O), np.float32)
    for g in range(G1):
        c = (p % 64) * F + 2 * g + p // 64
        out[:, g, :] = w1t[c]
    return out


def kernel(cin_inputs, w0, b0, w1, b1, _trace=False):
    from concourse.bass_utils import run_bass_kernel_spmd

    x = np.asarray(cin_inputs, dtype=np.float32)
    assert x.shape == (B_FULL, F, D)
    bf = ml_dtypes.bfloat16
    xt_all = np.ascontiguousarray(
        x.reshape(N_CORES, BS, F, D).transpose(0, 2, 1, 3)
    ).astype(bf).reshape(N_CORES, F, BS * D)
    xt_all = np.ascontiguousarray(np.tile(xt_all, (1, 4, 1)))

    w0_np = np.asarray(w0, dtype=np.float32)
    A0, w0fold = make_l0(w0_np)
    a0c = np.ascontiguousarray(
        np.tile(A0.reshape(F, NG0, 128), (4, 1, 1))
    ).astype(bf)
    w0fc = np.ascontiguousarray(
        w0fold.reshape(NG0, 128, O).transpose(1, 0, 2)
    ).astype(bf)
    w1gc = np.ascontiguousarray(make_w1g(w1)).astype(bf)
    b0c = np.asarray(b0, dtype=np.float32).reshape(O, 1).copy()
    b1c = np.asarray(b1, dtype=np.float32).reshape(O, 1).copy()

    nc = _get_nc()
    in_maps = []
    for i in range(N_CORES):
        in_maps.append(
            {
                "xt": xt_all[i],
                "a0": a0c, "w0f": w0fc, "w1g": w1gc,
                "b0": b0c, "b1": b1c,
            }
        )
    res = run_bass_kernel_spmd(nc, in_maps, core_ids=list(range(N_CORES)), trace=_trace)
    outs = []
    for r in res.results:
        o = np.concatenate([r["out0"], r["out1"]], axis=0).T
        outs.append(o)
    full = np.concatenate(outs, axis=0).astype(np.float32)
    if _trace:
        return full, res
    return full


# revision 11
# speedup vs baseline: 1.1905x; 1.1905x over previous
"""
CIN (Compressed Interaction Network) kernel for Trainium2, 8 NeuronCores.

Problem (hardcoded):
  x: [4096, 32, 64] fp32; w0: [128, 1024]; b0: [128]; w1: [128, 2048]; b1: [128]
  out: [4096, 192] = concat(relu(y0)[:, 64:], relu(y1)).sum(d)

Sharding: data parallel over batch, 512 samples/core, tokens t=(b,d), T=32768.

Key structure (per core, software-pipelined over 2048-token pairs):
  - L0 via polarization: x_h*x_f = ((x_h+x_f)^2 - x_h^2 - x_f^2)/2 ->
    496 upper-triangle sum-channels + 32 squares = 528 channels (vs 1024).
    Built on the PE as K=32 two-hot matmuls (4-way tile_position
    concurrency over the 4 x-copies); ScalarE evacuates with func=Square;
    contraction uses host-folded weights.
  - L1 z-channels are PERMUTED to (f, h) order so the broadcast operand is
    built from x (a pure input): xe_g[p, t] = x[2g + p//64, t] is
    DMA-expanded from HBM with a replicating access pattern - a pure
    prefetch with no upstream dependency (no hidden round-trip). The hidden
    side is one 2x-tiled SBUF copy per pair (hd2[p] = hidden[p % 64]).
    Contract weights are host-permuted to match.
  - z muls run in place on the expanded tiles (VectorE x14, GpSimd x2);
    contracts are group-major into 4 resident PSUM quarter tiles; y evac
    ScalarE Relu+bias; d-sums via log2 trees of strided VectorE adds.
  - Block P emits: xe prefetch for pair P, L0 of pair P, then muls +
    L1 contract of pair P-1, keeping every engine a full block ahead of
    its consumers.
"""

import sys

import numpy as np
import ml_dtypes

sys.path.insert(0, "/opt/trn_rl_repo")

B_FULL = 4096
N_CORES = 8
BS = B_FULL // N_CORES  # 512
F = 32
D = 64
T = BS * D
PAIR = 2048
O = 128
H1 = 64
G1 = 16
CH0 = 528
NG0 = 5

GPS_MULS = ()

_CACHE = {}


def _build_nc(BS=BS):
    import concourse.bass as bass
    import concourse.tile as tile
    from concourse import bacc, mybir

    T = BS * D
    NPAIR = T // PAIR
    SPP = PAIR // D

    bf16 = mybir.dt.bfloat16
    f32 = mybir.dt.float32
    Relu = mybir.ActivationFunctionType.Relu
    Square = mybir.ActivationFunctionType.Square

    nc = bacc.Bacc(None, target_bir_lowering=False)

    xt = nc.dram_tensor("xt", [128, T], bf16, kind="ExternalInput")
    a0 = nc.dram_tensor("a0", [128, NG0, 128], bf16, kind="ExternalInput")
    w0f = nc.dram_tensor("w0f", [128, NG0, O], bf16, kind="ExternalInput")
    w1g = nc.dram_tensor("w1g", [128, G1, O], bf16, kind="ExternalInput")
    b0 = nc.dram_tensor("b0", [O, 1], f32, kind="ExternalInput")
    b1 = nc.dram_tensor("b1", [O, 1], f32, kind="ExternalInput")
    out0 = nc.dram_tensor("out0", [O - H1, BS], f32, kind="ExternalOutput")
    out1 = nc.dram_tensor("out1", [O, BS], f32, kind="ExternalOutput")

    with tile.TileContext(nc) as tc:
        with (
            tc.tile_pool(name="singles", bufs=1) as singles,
            tc.tile_pool(name="xrp", bufs=2) as xrp,
            tc.tile_pool(name="s0p", bufs=6) as s0p,
            tc.tile_pool(name="xep", bufs=13) as xep,
            tc.tile_pool(name="hd2p", bufs=2) as hd2p,
            tc.tile_pool(name="y0sbp", bufs=2) as y0sbp,
            tc.tile_pool(name="y1sbp", bufs=2) as y1sbp,
            tc.tile_pool(name="bcps", bufs=2, space="PSUM") as bcps,
            tc.tile_pool(name="yqp", bufs=4, space="PSUM") as yqp,
        ):
            a0s = singles.tile([128, NG0, 128], bf16)
            w0fs = singles.tile([128, NG0, O], bf16)
            w1s = singles.tile([128, G1, O], bf16)
            b0s = singles.tile([O, 1], f32)
            b1s = singles.tile([O, 1], f32)
            oacc0 = singles.tile([O, BS], f32)
            oacc1 = singles.tile([O, BS], f32)

            nc.gpsimd.dma_start(out=a0s[:], in_=a0[:])
            nc.gpsimd.dma_start(out=w0fs[:], in_=w0f[:])
            nc.gpsimd.dma_start(out=w1s[:], in_=w1g[:])
            nc.gpsimd.dma_start(out=b0s[:], in_=b0[:])
            nc.gpsimd.dma_start(out=b1s[:], in_=b1[:])

            # x rows with a broadcast-expansion view for the xe prefetches
            xb = xt.rearrange("(a h) t -> a h t", a=1).broadcast_to((64, 128, T))

            st = {}

            def prefetch_xe(P):
                """xe_g[p, t] = x[2g + p//64, t] for all 16 groups (pure input)."""
                xes = []
                for R in range(8):
                    xe = xep.tile([128, 2, PAIR], bf16, name=f"xe{P}_{R}", tag="xe")
                    xes.append(xe)
                for g in range(G1):
                    src = xb[:, 2 * g : 2 * g + 2, P * PAIR : (P + 1) * PAIR]
                    src = src.rearrange("r h t -> h r t")
                    nc.gpsimd.dma_start(out=xes[g // 2][:, g % 2, :], in_=src)
                st[P] = {"xes": xes}

            def s1_load(P):
                xr = xrp.tile([128, PAIR], bf16, name=f"xr{P}", tag="xr")
                nc.gpsimd.dma_start(out=xr[:], in_=xt[:, P * PAIR : (P + 1) * PAIR])
                st[P]["xr"] = xr
                st[P]["s0"] = []

            def s1_build(P, G):
                xr = st[P]["xr"]
                psA = bcps.tile([128, 2, 512], f32, name=f"psA{P}_{G}", tag="bc")
                psB = bcps.tile([128, 2, 512], f32, name=f"psB{P}_{G}", tag="bc")
                for c in range(4):
                    ps = psA if c < 2 else psB
                    nc.tensor.matmul(
                        ps[:, c % 2, :],
                        a0s[32 * c : 32 * c + 32, G, :],
                        xr[32 * c : 32 * c + 32, 512 * c : 512 * (c + 1)],
                        start=True, stop=True, tile_position=(32 * c, 0),
                    )
                sg = s0p.tile([128, PAIR], bf16, name=f"s0_{P}_{G}", tag="s0")
                for half, ps in ((0, psA), (1, psB)):
                    dst = sg[:, 1024 * half : 1024 * (half + 1)].rearrange(
                        "p (j c) -> p j c", j=2
                    )
                    nc.scalar.activation(dst, ps[:], Square)
                st[P]["s0"].append(sg)

            def s2(P):
                """L0 contract + y0 evac + out0 tree + hd2 copies."""
                s0g = st[P]["s0"]
                y0sb = y0sbp.tile([128, PAIR], bf16, name=f"y0sb{P}", tag="y0sb")
                yq = [
                    yqp.tile([128, 512], f32, name=f"y0q{P}_{q}", tag="yq")
                    for q in range(4)
                ]
                for G in range(NG0):
                    for q in range(4):
                        nc.tensor.matmul(
                            yq[q][:], w0fs[:, G, :],
                            s0g[G][:, 512 * q : 512 * (q + 1)],
                            start=(G == 0), stop=(G == NG0 - 1),
                        )
                for q in range(4):
                    nc.scalar.activation(
                        y0sb[:, 512 * q : 512 * (q + 1)], yq[q][:], Relu, bias=b0s[:]
                    )
                hd2 = hd2p.tile([128, PAIR], bf16, name=f"hd2_{P}", tag="hd2")
                nc.gpsimd.dma_start(out=hd2[0:64, :], in_=y0sb[0:H1, :])
                nc.gpsimd.dma_start(out=hd2[64:128, :], in_=y0sb[0:H1, :])
                st[P]["hd2"] = hd2
                w = D // 2
                while w >= 1:
                    a = y0sb[H1:O, :].rearrange("p (b d) -> p b d", d=D)
                    dst = (
                        oacc0[H1:O, P * SPP : (P + 1) * SPP]
                        if w == 1
                        else a[:, :, 0:w]
                    )
                    nc.vector.tensor_add(dst, a[:, :, 0:w], a[:, :, w : 2 * w])
                    w //= 2

            def s3_muls(P):
                xes, hd2 = st[P]["xes"], st[P]["hd2"]
                for g in range(G1):
                    xeg = xes[g // 2][:, g % 2, :]
                    eng = nc.gpsimd if g in GPS_MULS else nc.vector
                    eng.tensor_mul(xeg, xeg, hd2[:])
                y1sb = y1sbp.tile([128, PAIR], bf16, name=f"y1sb{P}", tag="y1sb")
                yq = [
                    yqp.tile([128, 512], f32, name=f"y1q{P}_{q}", tag="yq")
                    for q in range(4)
                ]
                st[P]["y1sb"] = y1sb
                st[P]["yq"] = yq

            def s3_contract(P, glo, ghi):
                xes, yq = st[P]["xes"], st[P]["yq"]
                for g in range(glo, ghi):
                    for q in range(4):
                        nc.tensor.matmul(
                            yq[q][:], w1s[:, g, :],
                            xes[g // 2][:, g % 2, 512 * q : 512 * (q + 1)],
                            start=(g == 0), stop=(g == G1 - 1),
                        )

            def s3_fini(P):
                y1sb, yq = st[P]["y1sb"], st[P]["yq"]
                for q in range(4):
                    nc.scalar.activation(
                        y1sb[:, 512 * q : 512 * (q + 1)], yq[q][:], Relu, bias=b1s[:]
                    )
                w = D // 2
                while w >= 1:
                    a = y1sb[:].rearrange("p (b d) -> p b d", d=D)
                    dst = (
                        oacc1[:, P * SPP : (P + 1) * SPP] if w == 1 else a[:, :, 0:w]
                    )
                    nc.vector.tensor_add(dst, a[:, :, 0:w], a[:, :, w : 2 * w])
                    w //= 2
                del st[P]

            for P in range(NPAIR + 1):
                p1, p3 = P, P - 1
                if p1 < NPAIR:
                    prefetch_xe(p1)
                    s1_load(p1)
                    s1_build(p1, 0)
                    s1_build(p1, 1)
                if 0 <= p3:
                    s3_muls(p3)
                    s3_contract(p3, 0, 4)
                if p1 < NPAIR:
                    s1_build(p1, 2)
                if 0 <= p3:
                    s3_contract(p3, 4, 8)
                if p1 < NPAIR:
                    s1_build(p1, 3)
                if 0 <= p3:
                    s3_contract(p3, 8, 12)
                if p1 < NPAIR:
                    s1_build(p1, 4)
                if 0 <= p3:
                    s3_contract(p3, 12, 16)
                    s3_fini(p3)
                if p1 < NPAIR:
                    s2(p1)

            nc.gpsimd.dma_start(out=out0[:], in_=oacc0[H1:O, :])
            nc.gpsimd.dma_start(out=out1[:], in_=oacc1[:])

    nc.finalize()
    return nc


def _get_nc():
    if "nc" not in _CACHE:
        _CACHE["nc"] = _build_nc()
    return _CACHE["nc"]


def _l0_pairs():
    return [(h, f) for h in range(F) for f in range(h + 1, F)]


def make_l0(w0_np):
    """A0 build matrix [32, 640] and folded weights [640, 128] (zero-padded)."""
    pairs = _l0_pairs()
    A0 = np.zeros((F, NG0 * 128), np.float32)
    w0fold = np.zeros((NG0 * 128, O), np.float32)
    for k, (h, f) in enumerate(pairs):
        A0[h, k] = 1.0
        A0[f, k] = 1.0
        w0fold[k] = (w0_np[:, h * F + f] + w0_np[:, f * F + h]) / 2
    for h in range(F):
        k = 496 + h
        A0[h, k] = 1.0
        c = w0_np[:, h * F + h].copy()
        for f in range(F):
            if f != h:
                c -= 0.5 * (w0_np[:, h * F + f] + w0_np[:, f * F + h])
        w0fold[k] = c
    return A0, w0fold


def make_w1g(w1_np):
    """Permuted L1 weights [128, 16, 128]: channel (g, p) -> h*F + f with
    f = 2g + p//64, h = p%64."""
    w1t = np.ascontiguousarray(np.asarray(w1_np, dtype=np.float32).T)  # [2048, O]
    p = np.arange(128)
    out = np.empty((128, G1, O), np.float32)
    for g in range(G1):
        c = (p % 64) * F + 2 * g + p // 64
        out[:, g, :] = w1t[c]
    return out


def kernel(cin_inputs, w0, b0, w1, b1, _trace=False):
    from concourse.bass_utils import run_bass_kernel_spmd

    x = np.asarray(cin_inputs, dtype=np.float32)
    assert x.shape == (B_FULL, F, D)
    bf = ml_dtypes.bfloat16
    xt_all = np.ascontiguousarray(
        x.reshape(N_CORES, BS, F, D).transpose(0, 2, 1, 3)
    ).astype(bf).reshape(N_CORES, F, BS * D)
    xt_all = np.ascontiguousarray(np.tile(xt_all, (1, 4, 1)))

    w0_np = np.asarray(w0, dtype=np.float32)
    A0, w0fold = make_l0(w0_np)
    a0c = np.ascontiguousarray(
        np.tile(A0.reshape(F, NG0, 128), (4, 1, 1))
    ).astype(bf)
    w0fc = np.ascontiguousarray(
        w0fold.reshape(NG0, 128, O).transpose(1, 0, 2)
    ).astype(bf)
    w1gc = np.ascontiguousarray(make_w1g(w1)).astype(bf)
    b0c = np.asarray(b0, dtype=np.float32).reshape(O, 1).copy()
    b1c = np.asarray(b1, dtype=np.float32).reshape(O, 1).copy()

    nc = _get_nc()
    in_maps = []
    for i in range(N_CORES):
        in_maps.append(
            {
                "xt": xt_all[i],
                "a0": a0c, "w0f": w0fc, "w1g": w1gc,
                "b0": b0c, "b1": b1c,
            }
        )
    res = run_bass_kernel_spmd(nc, in_maps, core_ids=list(range(N_CORES)), trace=_trace)
    outs = []
    for r in res.results:
        o = np.concatenate([r["out0"], r["out1"]], axis=0).T
        outs.append(o)
    full = np.concatenate(outs, axis=0).astype(np.float32)
    if _trace:
        return full, res
    return full


# revision 12
# speedup vs baseline: 2.2602x; 1.8986x over previous
"""
CIN (Compressed Interaction Network) kernel for Trainium2, 8 NeuronCores.

Problem (hardcoded):
  x: [4096, 32, 64] fp32; w0: [128, 1024]; b0: [128]; w1: [128, 2048]; b1: [128]
  out: [4096, 192] = concat(relu(y0)[:, 64:], relu(y1)).sum(d)

Sharding: data parallel over batch, 512 samples/core, tokens t=(b,d), T=32768.

Key structure (per core, software-pipelined over 2048-token pairs):
  - L0 via polarization: x_h*x_f = ((x_h+x_f)^2 - x_h^2 - x_f^2)/2 ->
    496 upper-triangle sum-channels + 32 squares = 528 channels (vs 1024).
    Built on the PE as K=32 two-hot matmuls (4-way tile_position
    concurrency over the 4 x-copies); ScalarE evacuates with func=Square;
    contraction uses host-folded weights.
  - L1 z-channels are PERMUTED to (f, h) order so the broadcast operand is
    built from x (a pure input): xe_g[p, t] = x[2g + p//64, t] is
    DMA-expanded from HBM with a replicating access pattern - a pure
    prefetch with no upstream dependency (no hidden round-trip). The hidden
    side is one 2x-tiled SBUF copy per pair (hd2[p] = hidden[p % 64]).
    Contract weights are host-permuted to match.
  - z muls run in place on the expanded tiles (VectorE x14, GpSimd x2);
    contracts are group-major into 4 resident PSUM quarter tiles; y evac
    ScalarE Relu+bias; d-sums via log2 trees of strided VectorE adds.
  - Block P emits: xe prefetch for pair P, L0 of pair P, then muls +
    L1 contract of pair P-1, keeping every engine a full block ahead of
    its consumers.
"""

import sys

import numpy as np
import ml_dtypes

sys.path.insert(0, "/opt/trn_rl_repo")

B_FULL = 4096
N_CORES = 8
BS = B_FULL // N_CORES  # 512
F = 32
D = 64
T = BS * D
PAIR = 2048
O = 128
H1 = 64
G1 = 16
CH0 = 528
NG0 = 5

GPS_MULS = ()
DMA_G = list(range(12))   # groups delivered by plain DMA from host-expanded xe1h
PE_G = [12, 13, 14, 15]   # groups built on the PE from one-hot selects

_CACHE = {}


def _build_nc(BS=BS):
    import concourse.bass as bass
    import concourse.tile as tile
    from concourse import bacc, mybir

    T = BS * D
    NPAIR = T // PAIR
    SPP = PAIR // D

    bf16 = mybir.dt.bfloat16
    f32 = mybir.dt.float32
    Relu = mybir.ActivationFunctionType.Relu
    Square = mybir.ActivationFunctionType.Square

    nc = bacc.Bacc(None, target_bir_lowering=False)

    xt = nc.dram_tensor("xt", [128, T], bf16, kind="ExternalInput")
    xe1h = nc.dram_tensor("xe1h", [len(DMA_G) * 128, T], bf16, kind="ExternalInput")
    selx = nc.dram_tensor("selx", [128, len(PE_G), 128], bf16, kind="ExternalInput")
    a0 = nc.dram_tensor("a0", [128, NG0, 128], bf16, kind="ExternalInput")
    w0f = nc.dram_tensor("w0f", [128, NG0, O], bf16, kind="ExternalInput")
    w1g = nc.dram_tensor("w1g", [128, G1, O], bf16, kind="ExternalInput")
    b0 = nc.dram_tensor("b0", [O, 1], f32, kind="ExternalInput")
    b1 = nc.dram_tensor("b1", [O, 1], f32, kind="ExternalInput")
    out0 = nc.dram_tensor("out0", [O - H1, BS], f32, kind="ExternalOutput")
    out1 = nc.dram_tensor("out1", [O, BS], f32, kind="ExternalOutput")

    with tile.TileContext(nc) as tc:
        with (
            tc.tile_pool(name="singles", bufs=1) as singles,
            tc.tile_pool(name="xrp", bufs=2) as xrp,
            tc.tile_pool(name="s0p", bufs=6) as s0p,
            tc.tile_pool(name="xep", bufs=13) as xep,
            tc.tile_pool(name="hd2p", bufs=2) as hd2p,
            tc.tile_pool(name="y0sbp", bufs=2) as y0sbp,
            tc.tile_pool(name="y1sbp", bufs=2) as y1sbp,
            tc.tile_pool(name="bcps", bufs=2, space="PSUM") as bcps,
            tc.tile_pool(name="yqp", bufs=4, space="PSUM") as yqp,
        ):
            a0s = singles.tile([128, NG0, 128], bf16)
            selxs = singles.tile([128, len(PE_G), 128], bf16)
            w0fs = singles.tile([128, NG0, O], bf16)
            w1s = singles.tile([128, G1, O], bf16)
            b0s = singles.tile([O, 1], f32)
            b1s = singles.tile([O, 1], f32)
            oacc0 = singles.tile([O, BS], f32)
            oacc1 = singles.tile([O, BS], f32)

            nc.gpsimd.dma_start(out=a0s[:], in_=a0[:])
            nc.gpsimd.dma_start(out=selxs[:], in_=selx[:])
            nc.gpsimd.dma_start(out=w0fs[:], in_=w0f[:])
            nc.gpsimd.dma_start(out=w1s[:], in_=w1g[:])
            nc.gpsimd.dma_start(out=b0s[:], in_=b0[:])
            nc.gpsimd.dma_start(out=b1s[:], in_=b1[:])

            st = {}

            def prefetch_xe(P):
                """xe_g[p, t] = x[2g + p//64, t]; 12 groups via plain reads of
                the host-expanded xe1h, 4 groups PE-built later in the block."""
                xes = []
                for R in range(8):
                    xe = xep.tile([128, 2, PAIR], bf16, name=f"xe{P}_{R}", tag="xe")
                    xes.append(xe)
                for i, g in enumerate(DMA_G):
                    nc.gpsimd.dma_start(
                        out=xes[g // 2][:, g % 2, :],
                        in_=xe1h[128 * i : 128 * (i + 1), P * PAIR : (P + 1) * PAIR],
                    )
                st[P] = {"xes": xes}

            def build_xe_pe(P):
                """PE-built broadcast for PE_G: one-hot selects of x rows."""
                xr, xes = st[P]["xr"], st[P]["xes"]
                for k, g in enumerate(PE_G):
                    psA = bcps.tile([128, 2, 512], f32, name=f"xpsA{P}_{k}", tag="bc")
                    psB = bcps.tile([128, 2, 512], f32, name=f"xpsB{P}_{k}", tag="bc")
                    for c in range(4):
                        ps = psA if c < 2 else psB
                        nc.tensor.matmul(
                            ps[:, c % 2, :],
                            selxs[32 * c : 32 * c + 32, k, :],
                            xr[32 * c : 32 * c + 32, 512 * c : 512 * (c + 1)],
                            start=True, stop=True, tile_position=(32 * c, 0),
                        )
                    for half, ps in ((0, psA), (1, psB)):
                        dst = xes[g // 2][
                            :, g % 2, 1024 * half : 1024 * (half + 1)
                        ].rearrange("p (j c) -> p j c", j=2)
                        if k < 2:
                            nc.scalar.activation(
                                dst, ps[:], mybir.ActivationFunctionType.Copy
                            )
                        else:
                            nc.vector.tensor_copy(dst, ps[:])

            def s1_load(P):
                xr = xrp.tile([128, PAIR], bf16, name=f"xr{P}", tag="xr")
                nc.gpsimd.dma_start(out=xr[:], in_=xt[:, P * PAIR : (P + 1) * PAIR])
                st[P]["xr"] = xr
                st[P]["s0"] = []

            def s1_build(P, G):
                xr = st[P]["xr"]
                psA = bcps.tile([128, 2, 512], f32, name=f"psA{P}_{G}", tag="bc")
                psB = bcps.tile([128, 2, 512], f32, name=f"psB{P}_{G}", tag="bc")
                for c in range(4):
                    ps = psA if c < 2 else psB
                    nc.tensor.matmul(
                        ps[:, c % 2, :],
                        a0s[32 * c : 32 * c + 32, G, :],
                        xr[32 * c : 32 * c + 32, 512 * c : 512 * (c + 1)],
                        start=True, stop=True, tile_position=(32 * c, 0),
                    )
                sg = s0p.tile([128, PAIR], bf16, name=f"s0_{P}_{G}", tag="s0")
                for half, ps in ((0, psA), (1, psB)):
                    dst = sg[:, 1024 * half : 1024 * (half + 1)].rearrange(
                        "p (j c) -> p j c", j=2
                    )
                    nc.scalar.activation(dst, ps[:], Square)
                st[P]["s0"].append(sg)

            def s2(P):
                """L0 contract + y0 evac + out0 tree + hd2 copies."""
                s0g = st[P]["s0"]
                y0sb = y0sbp.tile([128, PAIR], bf16, name=f"y0sb{P}", tag="y0sb")
                yq = [
                    yqp.tile([128, 512], f32, name=f"y0q{P}_{q}", tag="yq")
                    for q in range(4)
                ]
                for G in range(NG0):
                    for q in range(4):
                        nc.tensor.matmul(
                            yq[q][:], w0fs[:, G, :],
                            s0g[G][:, 512 * q : 512 * (q + 1)],
                            start=(G == 0), stop=(G == NG0 - 1),
                        )
                for q in range(4):
                    nc.scalar.activation(
                        y0sb[:, 512 * q : 512 * (q + 1)], yq[q][:], Relu, bias=b0s[:]
                    )
                hd2 = hd2p.tile([128, PAIR], bf16, name=f"hd2_{P}", tag="hd2")
                nc.gpsimd.dma_start(out=hd2[0:64, :], in_=y0sb[0:H1, :])
                nc.gpsimd.dma_start(out=hd2[64:128, :], in_=y0sb[0:H1, :])
                st[P]["hd2"] = hd2
                w = D // 2
                while w >= 1:
                    a = y0sb[H1:O, :].rearrange("p (b d) -> p b d", d=D)
                    dst = (
                        oacc0[H1:O, P * SPP : (P + 1) * SPP]
                        if w == 1
                        else a[:, :, 0:w]
                    )
                    nc.vector.tensor_add(dst, a[:, :, 0:w], a[:, :, w : 2 * w])
                    w //= 2

            def s3_muls(P):
                xes, hd2 = st[P]["xes"], st[P]["hd2"]
                for g in range(G1):
                    xeg = xes[g // 2][:, g % 2, :]
                    eng = nc.gpsimd if g in GPS_MULS else nc.vector
                    eng.tensor_mul(xeg, xeg, hd2[:])
                y1sb = y1sbp.tile([128, PAIR], bf16, name=f"y1sb{P}", tag="y1sb")
                yq = [
                    yqp.tile([128, 512], f32, name=f"y1q{P}_{q}", tag="yq")
                    for q in range(4)
                ]
                st[P]["y1sb"] = y1sb
                st[P]["yq"] = yq

            def s3_contract(P, glo, ghi):
                xes, yq = st[P]["xes"], st[P]["yq"]
                for g in range(glo, ghi):
                    for q in range(4):
                        nc.tensor.matmul(
                            yq[q][:], w1s[:, g, :],
                            xes[g // 2][:, g % 2, 512 * q : 512 * (q + 1)],
                            start=(g == 0), stop=(g == G1 - 1),
                        )

            def s3_fini(P):
                y1sb, yq = st[P]["y1sb"], st[P]["yq"]
                for q in range(4):
                    nc.scalar.activation(
                        y1sb[:, 512 * q : 512 * (q + 1)], yq[q][:], Relu, bias=b1s[:]
                    )
                w = D // 2
                while w >= 1:
                    a = y1sb[:].rearrange("p (b d) -> p b d", d=D)
                    dst = (
                        oacc1[:, P * SPP : (P + 1) * SPP] if w == 1 else a[:, :, 0:w]
                    )
                    nc.vector.tensor_add(dst, a[:, :, 0:w], a[:, :, w : 2 * w])
                    w //= 2
                del st[P]

            for P in range(NPAIR + 1):
                p1, p3 = P, P - 1
                if p1 < NPAIR:
                    prefetch_xe(p1)
                    s1_load(p1)
                    s1_build(p1, 0)
                    s1_build(p1, 1)
                if 0 <= p3:
                    s3_muls(p3)
                    s3_contract(p3, 0, 4)
                if p1 < NPAIR:
                    s1_build(p1, 2)
                if 0 <= p3:
                    s3_contract(p3, 4, 8)
                if p1 < NPAIR:
                    s1_build(p1, 3)
                if 0 <= p3:
                    s3_contract(p3, 8, 12)
                if p1 < NPAIR:
                    s1_build(p1, 4)
                if 0 <= p3:
                    s3_contract(p3, 12, 16)
                    s3_fini(p3)
                if p1 < NPAIR:
                    build_xe_pe(p1)
                    s2(p1)

            nc.gpsimd.dma_start(out=out0[:], in_=oacc0[H1:O, :])
            nc.gpsimd.dma_start(out=out1[:], in_=oacc1[:])

    nc.finalize()
    return nc


def _get_nc():
    if "nc" not in _CACHE:
        _CACHE["nc"] = _build_nc()
    return _CACHE["nc"]


def _l0_pairs():
    return [(h, f) for h in range(F) for f in range(h + 1, F)]


def make_l0(w0_np):
    """A0 build matrix [32, 640] and folded weights [640, 128] (zero-padded)."""
    pairs = _l0_pairs()
    A0 = np.zeros((F, NG0 * 128), np.float32)
    w0fold = np.zeros((NG0 * 128, O), np.float32)
    for k, (h, f) in enumerate(pairs):
        A0[h, k] = 1.0
        A0[f, k] = 1.0
        w0fold[k] = (w0_np[:, h * F + f] + w0_np[:, f * F + h]) / 2
    for h in range(F):
        k = 496 + h
        A0[h, k] = 1.0
        c = w0_np[:, h * F + h].copy()
        for f in range(F):
            if f != h:
                c -= 0.5 * (w0_np[:, h * F + f] + w0_np[:, f * F + h])
        w0fold[k] = c
    return A0, w0fold


def make_xe1h(x_core_bf):
    """Host-expanded broadcast rows for DMA_G: [12*128, T]."""
    row = np.empty(len(DMA_G) * 128, np.int64)
    p = np.arange(128)
    for i, g in enumerate(DMA_G):
        row[128 * i : 128 * (i + 1)] = 2 * g + p // 64
    return np.ascontiguousarray(x_core_bf[row])


def make_selx():
    sel = np.zeros((128, len(PE_G), 128), np.float32)
    for k, g in enumerate(PE_G):
        for s in range(4):
            for m in range(128):
                sel[32 * s + 2 * g + m // 64, k, m] = 1.0
    return sel


def make_w1g(w1_np):
    """Permuted L1 weights [128, 16, 128]: channel (g, p) -> h*F + f with
    f = 2g + p//64, h = p%64."""
    w1t = np.ascontiguousarray(np.asarray(w1_np, dtype=np.float32).T)  # [2048, O]
    p = np.arange(128)
    out = np.empty((128, G1, O), np.float32)
    for g in range(G1):
        c = (p % 64) * F + 2 * g + p // 64
        out[:, g, :] = w1t[c]
    return out


def kernel(cin_inputs, w0, b0, w1, b1, _trace=False):
    from concourse.bass_utils import run_bass_kernel_spmd

    x = np.asarray(cin_inputs, dtype=np.float32)
    assert x.shape == (B_FULL, F, D)
    bf = ml_dtypes.bfloat16
    xt_all = np.ascontiguousarray(
        x.reshape(N_CORES, BS, F, D).transpose(0, 2, 1, 3)
    ).astype(bf).reshape(N_CORES, F, BS * D)
    xt_all = np.ascontiguousarray(np.tile(xt_all, (1, 4, 1)))

    w0_np = np.asarray(w0, dtype=np.float32)
    A0, w0fold = make_l0(w0_np)
    a0c = np.ascontiguousarray(
        np.tile(A0.reshape(F, NG0, 128), (4, 1, 1))
    ).astype(bf)
    w0fc = np.ascontiguousarray(
        w0fold.reshape(NG0, 128, O).transpose(1, 0, 2)
    ).astype(bf)
    w1gc = np.ascontiguousarray(make_w1g(w1)).astype(bf)
    selxc = make_selx().astype(bf)
    b0c = np.asarray(b0, dtype=np.float32).reshape(O, 1).copy()
    b1c = np.asarray(b1, dtype=np.float32).reshape(O, 1).copy()

    nc = _get_nc()
    in_maps = []
    for i in range(N_CORES):
        in_maps.append(
            {
                "xt": xt_all[i],
                "xe1h": make_xe1h(xt_all[i][0:F]),
                "a0": a0c, "w0f": w0fc, "w1g": w1gc, "selx": selxc,
                "b0": b0c, "b1": b1c,
            }
        )
    res = run_bass_kernel_spmd(nc, in_maps, core_ids=list(range(N_CORES)), trace=_trace)
    outs = []
    for r in res.results:
        o = np.concatenate([r["out0"], r["out1"]], axis=0).T
        outs.append(o)
    full = np.concatenate(outs, axis=0).astype(np.float32)
    if _trace:
        return full, res
    return full
